# revision 18
# baseline (speedup 1.0000x reference)
"""Trainium2 Bass kernel for a dense transformer block (B=2, T=2048, C=1024,
16 heads, causal attention with x64 score scale, MLP 4x), distributed over
8 NeuronCores.

Sharding: token-parallel.  Cores 0-3 take batch element 0, cores 4-7 batch
element 1.  Within a batch element the 16 query tiles of 128 tokens are dealt
round-robin (core j gets tiles j, j+4, j+8, j+12), which balances causal
attention cost and keeps the instruction stream identical across cores (SPMD).
K/V are computed redundantly per core for its whole batch element and kept
SBUF-resident (no HBM round-trip).

v2 layout: the K/V/Q projections and the attention for head-pair g are
interleaved per g so the tensor engine never idles long enough to re-arm the
HAM throttle.  LayerNorm gains/biases are folded into the weights host-side;
softmax runs streaming with chunk-local max and the global fixup + 1/den
folded into a diagonal matrix used as the transpose's moving operand.

Precision: q/k chain in float32r (scores reach +-1780, so exp(score) is
sensitive to absolute score error; fp32r keeps it ~1e-3).  V, Wo, W1, W2 and
the attention probabilities run in bf16.
"""
import numpy as np
import ml_dtypes

import concourse.bass as bass
import concourse.mybir as mybir
import concourse.tile as tile
from concourse.masks import make_identity
from concourse.vector_clock import ScopedClock
from concourse import bass_utils
from concourse.bass_utils import run_bass_kernel_spmd

_orig_run_command = bass_utils.run_command


def _run_command_ldwopt(cmd, **kw):
    return _orig_run_command(cmd, **kw)


P = 128
B, T, C = 2, 2048, 1024
NH, HD = 16, 64
NCT = C // P          # 8 channel tiles
NTC = T // 512        # 4 token 512-chunks per batch element
TOWN = 512            # own query tokens per core
NQT = TOWN // P       # 4 own query tiles
NG = NH // 2          # 8 head pairs
LN_EPS = 1e-5
FP = mybir.dt.float32
FR = mybir.dt.float32r
BF = mybir.dt.bfloat16
OP = mybir.AluOpType
AF = mybir.ActivationFunctionType
AX = mybir.AxisListType

# ---------------------------------------------------------------------------
# Workaround for walrus "Too many sync wait commands": most instruction
# structs in this compiler build accept only ~1 sync-wait.  Hoist overflow
# waits onto same-engine NoOps, and split the kernel-tail drain's
# global-clock waits across one drain instruction per clock domain.
# ---------------------------------------------------------------------------
_orig_commit_and_lower = tile.TileContext._commit_and_lower


def _ldw_sig(inst):
    a = inst.ins[0]
    try:
        return (a.memref, a.offset, str(a.ap), str(a.dtype))
    except AttributeError:
        return None


def _split_commit_and_lower(self, inst, original_block, old_bb_map, bb_to_exit_bb):
    # Drop weight reloads of the already-loaded stationary operand (the PE
    # queue executes in commit order; only Ldweights mutates the PE array).
    if isinstance(inst, mybir.InstLdweights):
        sig = _ldw_sig(inst)
        if sig is not None and sig == getattr(self, "_ldw_last_sig", None):
            si0 = inst.sync_info
            nop = mybir.InstNoOp(
                name=inst.name,
                sync_info=si0,
                bass_nofuse=True,
                engine=inst.engine,
            )
            return _split_commit_and_lower(self, nop, original_block,
                                           old_bb_map, bb_to_exit_bb)
        self._ldw_last_sig = sig
    si = getattr(inst, "sync_info", None)
    if (
        si is not None
        and si.on_wait
        and len(si.on_wait) > 1
        and type(inst).__name__.startswith("Inst")
    ):
        waits = list(si.on_wait)
        for w in waits[:-1]:
            nop = mybir.InstNoOp(
                name=self.nc.get_next_instruction_name(),
                sync_info=mybir.SyncInfo(on_wait=[w], on_update=[]),
                bass_nofuse=True,
                engine=inst.engine,
            )
            _orig_commit_and_lower(self, nop, original_block, old_bb_map, bb_to_exit_bb)
        inst.sync_info = mybir.SyncInfo(on_wait=waits[-1:], on_update=list(si.on_update))
    return _orig_commit_and_lower(self, inst, original_block, old_bb_map, bb_to_exit_bb)


def _split_drain_and_barrier(self, tick_clock, wait_clock):
    gc = tick_clock.global_clock
    entries = []
    scoped = gc.items() if hasattr(gc, "items") else [(None, gc)]
    for scope, vc in scoped:
        for proc in range(len(vc)):
            t = vc[proc]
            if t > 0:
                entries.append((scope, proc, t))
    if entries:
        for scope, proc, t in entries:
            drain_inst = self.nc.sync.drain()
            req = ScopedClock()
            req.require_at_least(scope, proc, t)
            wait_clock.add_sem_waits(drain_inst.ins, req)
    else:
        drain_inst = self.nc.sync.drain()
        wait_clock.add_sem_waits(
            drain_inst.ins, ScopedClock({None: tick_clock.global_clock})
        )
    self.nc.all_engine_barrier()
    assert self.sems is not None
    popped = self.nc._tile_sem_poison_stack.pop()
    assert popped is self._sem_poison
    self.nc.clear_and_free_semaphores(list(self.sems.allocated().values()))
    self.nc.all_engine_barrier()


def _apply_tile_patch():
    tile.TileContext._commit_and_lower = _split_commit_and_lower
    tile.TileContext._drain_and_barrier = _split_drain_and_barrier
    bass_utils.run_command = _run_command_ldwopt


# ---------------------------------------------------------------------------
# Host-side helpers
# ---------------------------------------------------------------------------

def _r12(a):
    """Round fp32 to float32r's grid (~11 mantissa bits) so on-device fp32r
    consumers see exactly representable values."""
    u = np.ascontiguousarray(a, np.float32).view(np.uint32).astype(np.uint64)
    u = (u + np.uint64(1 << 11)) & np.uint64(0xFFFFF000)
    return (u & np.uint64(0xFFFFFFFF)).astype(np.uint32).view(np.float32)


def _lhsT_tiles(w, km, mm):
    """[K, M] weight -> [M/128, K/128, 128, 128] lhsT tiles (w[m][k] block)."""
    k, m = w.shape
    return np.ascontiguousarray(
        w.reshape(km, P, mm, P).transpose(2, 0, 1, 3)
    )


def _bf16(a):
    return np.ascontiguousarray(a).astype(ml_dtypes.bfloat16)


def _coltile(b, n):
    """[n*128] bias vector -> [128, n] column tiles (partition-major)."""
    return np.ascontiguousarray(np.asarray(b, np.float32).reshape(n, P).T)


# ---------------------------------------------------------------------------
# Device kernel builder
# ---------------------------------------------------------------------------

def _build(nc):
    xT = nc.dram_tensor("xT", [C, T], FP, kind="ExternalInput").ap()
    xTo = nc.dram_tensor("xTo", [C, TOWN], FP, kind="ExternalInput").ap()
    wq = nc.dram_tensor("wq", [NCT, NCT, P, P], FP, kind="ExternalInput").ap()
    wk = nc.dram_tensor("wk", [NCT, NCT, P, P], FP, kind="ExternalInput").ap()
    wv = nc.dram_tensor("wv", [NCT, NCT, P, P], FP, kind="ExternalInput").ap()
    wo = nc.dram_tensor("wo", [NCT, NCT, P, P], BF, kind="ExternalInput").ap()
    w1 = nc.dram_tensor("w1", [32, NCT, P, P], BF, kind="ExternalInput").ap()
    w2 = nc.dram_tensor("w2", [NCT, 32, P, P], BF, kind="ExternalInput").ap()
    gb = nc.dram_tensor("gb", [P, NCT, 4], FP, kind="ExternalInput").ap()
    bqkv = nc.dram_tensor("bqkv", [P, NCT, 3], FP, kind="ExternalInput").ap()
    b1m = nc.dram_tensor("b1m", [P, 32], FP, kind="ExternalInput").ap()
    msk = nc.dram_tensor("msk", [NQT, P, 512], FP, kind="ExternalInput").ap()
    outT = nc.dram_tensor("outT", [C, TOWN], FP, kind="ExternalOutput").ap()

    with tile.TileContext(nc) as tc:
        _build_tc(nc, tc, xT, xTo, wq, wk, wv, wo, w1, w2, gb, bqkv, b1m, msk,
                  outT)
    return nc


def _ln_stats(nc, psum_st, psum_bc, ln_sb, const, src, sl, ones_col, ones_row,
              eps_t, nb=2):
    """Channel-major LN stats for src[:, :, sl] ([128, NCT, 512] fp32r).
    Returns (rb, mb): [128, 512] fp32r broadcast tiles with
    rb = 1/std per token, mb = mean/std per token."""
    ssum = psum_st.tile([1, 512], FP, tag="ssum", bufs=1)
    ssq = psum_st.tile([1, 512], FP, tag="ssq", bufs=1)
    for ct in range(NCT):
        nc.tensor.matmul(ssum[:], ones_col[:], src[:, ct, sl],
                         start=(ct == 0), stop=(ct == NCT - 1))
    for ct in range(NCT):
        sq = ln_sb.tile([P, 512], FR, tag="sq", bufs=nb)
        nc.scalar.activation(sq[:], src[:, ct, sl], AF.Square)
        nc.tensor.matmul(ssq[:], ones_col[:], sq[:],
                         start=(ct == 0), stop=(ct == NCT - 1))
    mean = ln_sb.tile([1, 512], FP, tag="mean", bufs=nb)
    msq = ln_sb.tile([1, 512], FP, tag="msq", bufs=nb)
    nc.scalar.mul(mean[:], ssum[:], 1.0 / C)
    nc.scalar.mul(msq[:], ssq[:], 1.0 / C)
    var = ln_sb.tile([1, 512], FP, tag="var", bufs=nb)
    nc.vector.tensor_tensor(var[:], mean[:], mean[:], op=OP.mult)
    nc.vector.tensor_tensor(var[:], msq[:], var[:], op=OP.subtract)
    sd = ln_sb.tile([1, 512], FP, tag="sd", bufs=nb)
    nc.scalar.activation(sd[:], var[:], AF.Sqrt, bias=eps_t[0:1, :])
    rstd_f = ln_sb.tile([1, 512], FP, tag="rstdf", bufs=nb)
    nc.vector.reciprocal(rstd_f[:], sd[:])
    rstd = ln_sb.tile([1, 512], FR, tag="rstd", bufs=nb)
    nc.vector.tensor_copy(rstd[:], rstd_f[:])
    mrstd = ln_sb.tile([1, 512], FR, tag="mrstd", bufs=nb)
    nc.vector.tensor_tensor(mrstd[:], mean[:], rstd_f[:], op=OP.mult)
    rb_ps = psum_bc.tile([P, 512], FP, tag="bcps", bufs=1, name="rb_ps")
    nc.tensor.matmul(rb_ps[:], ones_row[:], rstd[:], start=True, stop=True)
    rb = ln_sb.tile([P, 512], FR, tag="rb", bufs=nb)
    nc.scalar.copy(rb[:], rb_ps[:])
    mb_ps = psum_bc.tile([P, 512], FP, tag="bcps", bufs=1, name="mb_ps")
    nc.tensor.matmul(mb_ps[:], ones_row[:], mrstd[:], start=True, stop=True)
    mb = ln_sb.tile([P, 512], FR, tag="mb", bufs=nb)
    nc.scalar.copy(mb[:], mb_ps[:])
    return rb, mb


def _build_tc(nc, tc, xT, xTo, wq, wk, wv, wo, w1, w2, gb, bqkv, b1m, msk,
              outT):
    const_cm = tc.tile_pool(name="const", bufs=1)
    const = const_cm.__enter__()
    ident = const.tile([P, P], BF)
    make_identity(nc, ident[:])
    ones_col = const.tile([P, 1], FR)
    nc.any.memset(ones_col[:].bitcast(FP), 1.0)
    ones_row = const.tile([1, P], FR)
    nc.any.memset(ones_row[:].bitcast(FP), 1.0)
    eps_t = const.tile([P, 1], FP)
    nc.any.memset(eps_t[:], LN_EPS)
    gb_t = const.tile([P, NCT, 4], FP)
    nc.sync.dma_start(gb_t[:], gb)
    bqkv_t = const.tile([P, NCT, 3], FP)
    nc.sync.dma_start(bqkv_t[:], bqkv)
    b1m_t = const.tile([P, 32], FP)
    nc.sync.dma_start(b1m_t[:], b1m)
    mask_t = const.tile([P, NQT, 512], FP)
    nc.sync.dma_start(mask_t[:], msk.rearrange("i p m -> p i m"))

    g1c, b1c = gb_t[:, :, 0], gb_t[:, :, 1]
    g2c, b2c = gb_t[:, :, 2], gb_t[:, :, 3]
    bq_c, bk_c, bv_c = bqkv_t[:, :, 0], bqkv_t[:, :, 1], bqkv_t[:, :, 2]

    pers_cm = tc.tile_pool(name="pers", bufs=1)
    pers = pers_cm.__enter__()
    xn_own = pers.tile([P, NCT, TOWN], FR)    # LN1 pre-gb, own tokens
    xn_gb = pers.tile([P, NCT, TOWN], FR)     # LN1 with g1/b1, own tokens
    out_t = pers.tile([P, NCT, TOWN], BF)     # attention output

    xnpool_cm = tc.tile_pool(name="xnpool", bufs=1)
    xnpool = xnpool_cm.__enter__()
    xn_t = xnpool.tile([P, NCT, T], FR)       # 8 MB: x^T -> ln1 pre-gb in place
    nc.sync.dma_start(xn_t[:], xT.rearrange("(ct p) t -> p ct t", p=P).bitcast(FR))
    nc.sync.dma_start(xn_own[:], xTo.rearrange("(ct p) t -> p ct t", p=P).bitcast(FR))

    # ---------------- LN1 (own tokens first, then the full batch) --------
    with tc.tile_pool(name="ln_sb", bufs=1) as ln_sb, \
         tc.tile_pool(name="ps_st", bufs=1, space="PSUM") as ps_st, \
         tc.tile_pool(name="ps_bc", bufs=1, space="PSUM") as ps_bc:
        rb, mb = _ln_stats(nc, ps_st, ps_bc, ln_sb, const, xn_own,
                           slice(0, 512), ones_col, ones_row, eps_t)
        for ct in range(NCT):
            t1 = ln_sb.tile([P, 512], FR, tag="t1", bufs=2)
            nc.vector.tensor_tensor(t1[:], xn_own[:, ct, :], rb[:], op=OP.mult)
            nc.vector.tensor_tensor(xn_own[:, ct, :], t1[:], mb[:], op=OP.subtract)
            nc.vector.tensor_scalar(
                xn_gb[:, ct], xn_own[:, ct], g1c[:, ct:ct + 1],
                b1c[:, ct:ct + 1], op0=OP.mult, op1=OP.add,
            )
        for ch in range(NTC):
            sl = slice(ch * 512, (ch + 1) * 512)
            rb, mb = _ln_stats(nc, ps_st, ps_bc, ln_sb, const, xn_t, sl,
                               ones_col, ones_row, eps_t)
            for ct in range(NCT):
                t1 = ln_sb.tile([P, 512], FR, tag="t1", bufs=2)
                nc.vector.tensor_tensor(t1[:], xn_t[:, ct, sl], rb[:], op=OP.mult)
                nc.vector.tensor_tensor(xn_t[:, ct, sl], t1[:], mb[:], op=OP.subtract)

    # ---------------- Interleaved projections + attention ----------------
    with tc.tile_pool(name="wpool", bufs=1) as wpool, \
         tc.tile_pool(name="kpool", bufs=2) as kpool, \
         tc.tile_pool(name="vpool", bufs=2) as vpool, \
         tc.tile_pool(name="qpool", bufs=2) as qpool, \
         tc.tile_pool(name="stpool", bufs=2) as stpool, \
         tc.tile_pool(name="attpool", bufs=1) as attpool, \
         tc.tile_pool(name="attsm", bufs=3) as attsm, \
         tc.tile_pool(name="psP", bufs=2, space="PSUM") as psP, \
         tc.tile_pool(name="psS", bufs=2, space="PSUM") as psS, \
         tc.tile_pool(name="psT", bufs=2, space="PSUM") as psT, \
         tc.tile_pool(name="psAV", bufs=2, space="PSUM") as psAV:
        for g in range(NG):
            # --- K projection for head pair g (all T tokens) ---
            wk_t = wpool.tile([P, NCT, P], FR, tag="wkt", bufs=2)
            nc.sync.dma_start(wk_t[:], wk[g].rearrange("k p m -> p k m").bitcast(FR))
            k_g = kpool.tile([P, T], FR, tag="kg")
            for ch in range(NTC):
                sl = slice(ch * 512, (ch + 1) * 512)
                ps = psP.tile([P, 512], FP, tag="psP")
                for k in range(NCT):
                    nc.tensor.matmul(ps[:], wk_t[:, k], xn_t[:, k, sl],
                                     start=(k == 0), stop=(k == NCT - 1))
                nc.vector.tensor_scalar(k_g[:, sl], ps[:], bk_c[:, g:g + 1], None,
                                        op0=OP.add)
            # --- V projection for head pair g, transposed to token-major ---
            wv_t = wpool.tile([P, NCT, P], FR, tag="wvt", bufs=2)
            nc.sync.dma_start(wv_t[:], wv[g].rearrange("k p m -> p k m").bitcast(FR))
            v_g = vpool.tile([P, T // P, P], BF, tag="vg")
            for ch in range(NTC):
                sl = slice(ch * 512, (ch + 1) * 512)
                ps = psP.tile([P, 512], FP, tag="psP")
                for k in range(NCT):
                    nc.tensor.matmul(ps[:], wv_t[:, k], xn_t[:, k, sl],
                                     start=(k == 0), stop=(k == NCT - 1))
                st = stpool.tile([P, 512], BF, tag="vst")
                nc.vector.tensor_scalar(st[:], ps[:], bv_c[:, g:g + 1], None,
                                        op0=OP.add)
                pst = psT.tile([P, 512], BF, tag="tps", name="vtp")
                for b4 in range(4):
                    nc.tensor.transpose(pst[:, b4 * P:(b4 + 1) * P],
                                        st[:, b4 * P:(b4 + 1) * P], ident[:])
                ev = v_g[:, ch * 4:(ch + 1) * 4, :].rearrange("p n d -> p (n d)")
                nc.scalar.copy(ev, pst[:])
            # --- Q projection for head pair g (own tokens, x64*g1 folded) ---
            wq_t = wpool.tile([P, NCT, P], FR, tag="wqt", bufs=2)
            nc.sync.dma_start(wq_t[:], wq[g].rearrange("k p m -> p k m").bitcast(FR))
            q_g = qpool.tile([P, TOWN], FR, tag="qg")
            ps = psP.tile([P, 512], FP, tag="psP")
            for k in range(NCT):
                nc.tensor.matmul(ps[:], wq_t[:, k], xn_own[:, k, :],
                                 start=(k == 0), stop=(k == NCT - 1))
            nc.vector.tensor_scalar(q_g[:], ps[:], bq_c[:, g:g + 1], None,
                                    op0=OP.add)

            # --- attention for head pair g ---
            for i in range(NQT):
                nch = i + 1
                attT = attpool.tile([P, T // P, 2 * P], BF, tag="attT", bufs=2)
                for h2 in range(2):
                    pb = h2 * 64
                    q_sl = q_g[pb:pb + 64, i * P:(i + 1) * P]
                    att = attpool.tile([P, T], BF, tag=f"att{h2}", bufs=2)
                    nmx = attsm.tile([P, NQT], FP, tag=f"nmx{h2}")
                    dsum = attsm.tile([P, NQT], FP, tag=f"dsum{h2}")
                    for kk in range(nch):
                        ps_s = psS.tile([P, 512], FP, tag="sps")
                        nc.tensor.matmul(ps_s[:], q_sl,
                                         k_g[pb:pb + 64, kk * 512:(kk + 1) * 512],
                                         start=True, stop=True)
                        if kk == i:
                            nc.vector.tensor_tensor(ps_s[:], ps_s[:],
                                                    mask_t[:, i, :], op=OP.add)
                        nc.vector.tensor_reduce(nmx[:, kk:kk + 1], ps_s[:],
                                                axis=AX.X, op=OP.max,
                                                negate=True)
                        nc.scalar.activation(att[:, kk * 512:(kk + 1) * 512],
                                             ps_s[:], AF.Exp,
                                             bias=nmx[:, kk:kk + 1],
                                             accum_out=dsum[:, kk:kk + 1])
                    # global max fixup + 1/den, folded into per-chunk scales
                    nM = attsm.tile([P, 1], FP, tag=f"nM{h2}")
                    nc.vector.tensor_reduce(nM[:], nmx[:, 0:nch], axis=AX.X,
                                            op=OP.min)
                    diff = attsm.tile([P, NQT], FP, tag=f"diff{h2}")
                    nc.vector.tensor_scalar(diff[:, 0:nch], nmx[:, 0:nch],
                                            nM[:], None, op0=OP.subtract)
                    cvec = attsm.tile([P, NQT], FP, tag=f"cvec{h2}")
                    nc.scalar.activation(cvec[:, 0:nch], diff[:, 0:nch], AF.Exp,
                                         scale=-1.0)
                    wden = attsm.tile([P, NQT], FP, tag=f"wden{h2}")
                    nc.vector.tensor_tensor(wden[:, 0:nch], dsum[:, 0:nch],
                                            cvec[:, 0:nch], op=OP.mult)
                    den = attsm.tile([P, 1], FP, tag=f"den{h2}")
                    nc.vector.tensor_reduce(den[:], wden[:, 0:nch], axis=AX.X,
                                            op=OP.add)
                    rden = attsm.tile([P, 1], FP, tag=f"rden{h2}")
                    nc.vector.reciprocal(rden[:], den[:])
                    svec = attsm.tile([P, NQT], FP, tag=f"svec{h2}")
                    nc.vector.tensor_scalar_mul(svec[:, 0:nch], cvec[:, 0:nch],
                                                rden[:])
                    # scale each chunk by svec_kk, then transpose
                    sceng = nc.vector if h2 == 0 else nc.gpsimd
                    for kk in range(nch):
                        csl = slice(kk * 512, (kk + 1) * 512)
                        sceng.tensor_scalar_mul(att[:, csl], att[:, csl],
                                                svec[:, kk:kk + 1])
                        ps_t = psT.tile([P, 512], BF, tag="tps")
                        for b4 in range(4):
                            blk = kk * 4 + b4
                            nc.tensor.transpose(ps_t[:, b4 * P:(b4 + 1) * P],
                                                att[:, blk * P:(blk + 1) * P],
                                                ident[:])
                        ev = attT[:, kk * 4:(kk + 1) * 4, h2 * P:(h2 + 1) * P]
                        src = ps_t[:].rearrange("p (n d) -> p n d", n=4)
                        if kk % 2 == 0:
                            nc.scalar.copy(ev, src)
                        else:
                            nc.vector.tensor_copy(ev, src)
                # AV: both heads' columns in one moving operand
                ps_av = psAV.tile([P, 2 * P], FP, tag="avps")
                for blk in range(nch * 4):
                    nc.tensor.matmul(
                        ps_av[:], v_g[:, blk, :], attT[:, blk, :],
                        start=(blk == 0), stop=(blk == nch * 4 - 1))
                nc.vector.tensor_copy(out_t[0:64, g, i * P:(i + 1) * P],
                                      ps_av[0:64, 0:P])
                nc.vector.tensor_copy(out_t[64:128, g, i * P:(i + 1) * P],
                                      ps_av[64:128, P:2 * P])

    xnpool_cm.__exit__(None, None, None)

    # ---------------- Tail: out-proj, LN2, MLP ---------------------------
    with tc.tile_pool(name="wpoolC", bufs=1) as wpoolC, \
         tc.tile_pool(name="ln_sbC", bufs=1) as ln_sbC, \
         tc.tile_pool(name="hpool", bufs=1) as hpool, \
         tc.tile_pool(name="apool", bufs=1) as apool, \
         tc.tile_pool(name="opool", bufs=2) as opool, \
         tc.tile_pool(name="psC", bufs=3, space="PSUM") as psC, \
         tc.tile_pool(name="psC_st", bufs=1, space="PSUM") as psC_st, \
         tc.tile_pool(name="psC_bc", bufs=1, space="PSUM") as psC_bc:
        h_t = hpool.tile([P, NCT, TOWN], FR)
        h2p = hpool.tile([P, NCT, TOWN], BF)     # LN2 pre-gb (for W1, g2 folded)
        h2gb = hpool.tile([P, NCT, TOWN], BF)    # LN2 with g2/b2 (residual)
        for m in range(NCT):
            wo_t = wpoolC.tile([P, NCT, P], BF, tag="wot", bufs=2)
            nc.sync.dma_start(wo_t[:], wo[m].rearrange("k p m -> p k m"))
            ps = psC.tile([P, 512], FP, tag="psC")
            for k in range(NCT):
                nc.tensor.matmul(ps[:], wo_t[:, k], out_t[:, k, :],
                                 start=(k == 0), stop=(k == NCT - 1))
            nc.vector.tensor_tensor(h_t[:, m], ps[:], xn_gb[:, m], op=OP.add)

        rb, mb = _ln_stats(nc, psC_st, psC_bc, ln_sbC, const, h_t,
                           slice(0, 512), ones_col, ones_row, eps_t, nb=1)
        for ct in range(NCT):
            t1 = ln_sbC.tile([P, 512], FR, tag="t1", bufs=2)
            nc.vector.tensor_tensor(t1[:], h_t[:, ct, :], rb[:], op=OP.mult)
            nc.vector.tensor_tensor(h2p[:, ct], t1[:], mb[:], op=OP.subtract)
            nc.vector.tensor_scalar(
                h2gb[:, ct], h2p[:, ct], g2c[:, ct:ct + 1], b2c[:, ct:ct + 1],
                op0=OP.mult, op1=OP.add,
            )

        a_t = apool.tile([P, 32, TOWN], BF)
        for m in range(32):
            w1_t = wpoolC.tile([P, NCT, P], BF, tag="w1t", bufs=3)
            nc.sync.dma_start(w1_t[:], w1[m].rearrange("k p m -> p k m"))
            ps = psC.tile([P, 512], FP, tag="psC")
            for k in range(NCT):
                nc.tensor.matmul(ps[:], w1_t[:, k], h2p[:, k, :],
                                 start=(k == 0), stop=(k == NCT - 1))
            nc.vector.tensor_scalar(a_t[:, m], ps[:], b1m_t[:, m:m + 1], 0.0,
                                    op0=OP.add, op1=OP.max)

        for m in range(NCT):
            w2_t = wpoolC.tile([P, 32, P], BF, tag="w2t", bufs=2)
            nc.sync.dma_start(w2_t[:], w2[m].rearrange("k p m -> p k m"))
            ps = psC.tile([P, 512], FP, tag="psC")
            for k in range(32):
                nc.tensor.matmul(ps[:], w2_t[:, k], a_t[:, k, :],
                                 start=(k == 0), stop=(k == 31))
            o_m = opool.tile([P, 512], FP, tag="om")
            nc.vector.tensor_tensor(o_m[:], ps[:], h2gb[:, m], op=OP.add)
            nc.sync.dma_start(outT[m * P:(m + 1) * P, :], o_m[:])

    pers_cm.__exit__(None, None, None)
    const_cm.__exit__(None, None, None)


# ---------------------------------------------------------------------------
# Public entry point
# ---------------------------------------------------------------------------
_cache = {}


def _get_nc():
    if "nc" not in _cache:
        _apply_tile_patch()
        nc = bass.Bass("TRN2", target_bir_lowering=False, debug=False,
                       num_devices=8)
        _build(nc)
        _cache["nc"] = nc
    return _cache["nc"]


def run(inputs, trace=False):
    x = np.asarray(inputs["x"], np.float32)
    Wk = np.asarray(inputs["Wk"], np.float32)
    Wq = np.asarray(inputs["Wq"], np.float32)
    Wv = np.asarray(inputs["Wv"], np.float32)
    Wo = np.asarray(inputs["Wo"], np.float32)
    W1 = np.asarray(inputs["W1"], np.float32)
    W2 = np.asarray(inputs["W2"], np.float32)
    g1 = np.asarray(inputs["g1"], np.float32)
    b1 = np.asarray(inputs["b1"], np.float32)
    g2 = np.asarray(inputs["g2"], np.float32)
    b2 = np.asarray(inputs["b2"], np.float32)

    # Fold LN gains into the weights; LN biases become per-output-column
    # bias rows added at PSUM eviction.
    wq_t = _r12(_lhsT_tiles(_r12(g1[:, None] * Wq * float(HD)), NCT, NCT))
    wk_t = _r12(_lhsT_tiles(_r12(g1[:, None] * Wk), NCT, NCT))
    wv_t = _r12(_lhsT_tiles(_r12(g1[:, None] * Wv), NCT, NCT))
    wo_t = _bf16(_lhsT_tiles(Wo, NCT, NCT))
    w1_t = _bf16(_lhsT_tiles(g2[:, None] * W1, NCT, 32))
    w2_t = _bf16(_lhsT_tiles(W2, 32, NCT))
    bq = _coltile(float(HD) * (b1 @ Wq), NCT)
    bk = _coltile(b1 @ Wk, NCT)
    bv = _coltile(b1 @ Wv, NCT)
    bqkv = np.ascontiguousarray(np.stack([bq, bk, bv], axis=-1))  # [P, NCT, 3]
    b1m_h = _coltile(b2 @ W1, 32)
    gbh = np.stack(
        [g1.reshape(NCT, P).T, b1.reshape(NCT, P).T,
         g2.reshape(NCT, P).T, b2.reshape(NCT, P).T], axis=-1
    ).astype(np.float32)  # [P, NCT, 4]

    in_maps = []
    own_tokens_by_core = []
    for c in range(8):
        b = c // 4
        j = c % 4
        tiles = [j + 4 * i for i in range(NQT)]
        toks = np.concatenate([np.arange(t * P, (t + 1) * P) for t in tiles])
        own_tokens_by_core.append((b, toks))
        xT_full = _r12(np.ascontiguousarray(x[b].T))
        xT_own = _r12(np.ascontiguousarray(x[b][toks].T))
        mask = np.zeros((NQT, P, 512), np.float32)
        for i in range(NQT):
            t0 = (j + 4 * i) * P
            Ei = (i + 1) * 512
            cols = (Ei - 512) + np.arange(512)
            rows = t0 + np.arange(P)
            mask[i] = np.where(cols[None, :] <= rows[:, None], 0.0, -1.0e30)
        in_maps.append({
            "xT": xT_full, "xTo": xT_own,
            "wq": wq_t, "wk": wk_t, "wv": wv_t, "wo": wo_t,
            "w1": w1_t, "w2": w2_t, "gb": gbh,
            "bqkv": bqkv, "b1m": b1m_h, "msk": mask,
        })

    nc = _get_nc()
    res = run_bass_kernel_spmd(nc, in_maps, core_ids=list(range(8)),
                               trace=trace)

    out = np.empty((B, T, C), np.float32)
    for c in range(8):
        b, toks = own_tokens_by_core[c]
        out[b, toks, :] = res.results[c]["outT"].T
    return out, res


def kernel(**inputs):
    out, _ = run(inputs, trace=False)
    return out


# revision 20
# speedup vs baseline: 1.7535x; 1.7535x over previous
"""Trainium2 Bass kernel for a dense transformer block (B=2, T=2048, C=1024,
16 heads, causal attention with x64 score scale, MLP 4x), distributed over
8 NeuronCores.

Sharding: token-parallel.  Cores 0-3 take batch element 0, cores 4-7 batch
element 1.  Within a batch element the 16 query tiles of 128 tokens are dealt
round-robin (core j gets tiles j, j+4, j+8, j+12), which balances causal
attention cost and keeps the instruction stream identical across cores (SPMD).
K/V are computed redundantly per core for its whole batch element and kept
SBUF-resident (no HBM round-trip).

v2 layout: the K/V/Q projections and the attention for head-pair g are
interleaved per g so the tensor engine never idles long enough to re-arm the
HAM throttle.  LayerNorm gains/biases are folded into the weights host-side;
softmax runs streaming with chunk-local max and the global fixup + 1/den
folded into a diagonal matrix used as the transpose's moving operand.

Precision: q/k chain in float32r (scores reach +-1780, so exp(score) is
sensitive to absolute score error; fp32r keeps it ~1e-3).  V, Wo, W1, W2 and
the attention probabilities run in bf16.
"""
import numpy as np
import ml_dtypes

import concourse.bass as bass
import concourse.mybir as mybir
import concourse.tile as tile
from concourse.masks import make_identity
from concourse.vector_clock import ScopedClock
from concourse import bass_utils
from concourse.bass_utils import run_bass_kernel_spmd

_orig_run_command = bass_utils.run_command


def _run_command_ldwopt(cmd, **kw):
    return _orig_run_command(cmd, **kw)


P = 128
B, T, C = 2, 2048, 1024
NH, HD = 16, 64
NCT = C // P          # 8 channel tiles
NTC = T // 512        # 4 token 512-chunks per batch element
TOWN = 512            # own query tokens per core
NQT = TOWN // P       # 4 own query tiles
NG = NH // 2          # 8 head pairs
LN_EPS = 1e-5
FP = mybir.dt.float32
FR = mybir.dt.float32r
BF = mybir.dt.bfloat16
OP = mybir.AluOpType
AF = mybir.ActivationFunctionType
AX = mybir.AxisListType

# ---------------------------------------------------------------------------
# Workaround for walrus "Too many sync wait commands": most instruction
# structs in this compiler build accept only ~1 sync-wait.  Hoist overflow
# waits onto same-engine NoOps, and split the kernel-tail drain's
# global-clock waits across one drain instruction per clock domain.
# ---------------------------------------------------------------------------
_orig_commit_and_lower = tile.TileContext._commit_and_lower


def _ldw_sig(inst):
    a = inst.ins[0]
    try:
        return (a.memref, a.offset, str(a.ap), str(a.dtype))
    except AttributeError:
        return None


def _split_commit_and_lower(self, inst, original_block, old_bb_map, bb_to_exit_bb):
    # Drop weight reloads of the already-loaded stationary operand (the PE
    # queue executes in commit order; only Ldweights mutates the PE array).
    if isinstance(inst, mybir.InstLdweights):
        sig = _ldw_sig(inst)
        if sig is not None and sig == getattr(self, "_ldw_last_sig", None):
            si0 = inst.sync_info
            nop = mybir.InstNoOp(
                name=inst.name,
                sync_info=si0,
                bass_nofuse=True,
                engine=inst.engine,
            )
            return _split_commit_and_lower(self, nop, original_block,
                                           old_bb_map, bb_to_exit_bb)
        self._ldw_last_sig = sig
    si = getattr(inst, "sync_info", None)
    if (
        si is not None
        and si.on_wait
        and len(si.on_wait) > 1
        and type(inst).__name__.startswith("Inst")
    ):
        waits = list(si.on_wait)
        for w in waits[:-1]:
            nop = mybir.InstNoOp(
                name=self.nc.get_next_instruction_name(),
                sync_info=mybir.SyncInfo(on_wait=[w], on_update=[]),
                bass_nofuse=True,
                engine=inst.engine,
            )
            _orig_commit_and_lower(self, nop, original_block, old_bb_map, bb_to_exit_bb)
        inst.sync_info = mybir.SyncInfo(on_wait=waits[-1:], on_update=list(si.on_update))
    return _orig_commit_and_lower(self, inst, original_block, old_bb_map, bb_to_exit_bb)


def _split_drain_and_barrier(self, tick_clock, wait_clock):
    gc = tick_clock.global_clock
    entries = []
    scoped = gc.items() if hasattr(gc, "items") else [(None, gc)]
    for scope, vc in scoped:
        for proc in range(len(vc)):
            t = vc[proc]
            if t > 0:
                entries.append((scope, proc, t))
    if entries:
        for scope, proc, t in entries:
            drain_inst = self.nc.sync.drain()
            req = ScopedClock()
            req.require_at_least(scope, proc, t)
            wait_clock.add_sem_waits(drain_inst.ins, req)
    else:
        drain_inst = self.nc.sync.drain()
        wait_clock.add_sem_waits(
            drain_inst.ins, ScopedClock({None: tick_clock.global_clock})
        )
    self.nc.all_engine_barrier()
    assert self.sems is not None
    popped = self.nc._tile_sem_poison_stack.pop()
    assert popped is self._sem_poison
    self.nc.clear_and_free_semaphores(list(self.sems.allocated().values()))
    self.nc.all_engine_barrier()


def _apply_tile_patch():
    tile.TileContext._commit_and_lower = _split_commit_and_lower
    tile.TileContext._drain_and_barrier = _split_drain_and_barrier
    bass_utils.run_command = _run_command_ldwopt


# ---------------------------------------------------------------------------
# Host-side helpers
# ---------------------------------------------------------------------------

def _r12(a):
    """Round fp32 to float32r's grid (~11 mantissa bits) so on-device fp32r
    consumers see exactly representable values."""
    u = np.ascontiguousarray(a, np.float32).view(np.uint32).astype(np.uint64)
    u = (u + np.uint64(1 << 11)) & np.uint64(0xFFFFF000)
    return (u & np.uint64(0xFFFFFFFF)).astype(np.uint32).view(np.float32)


def _lhsT_tiles(w, km, mm):
    """[K, M] weight -> [M/128, K/128, 128, 128] lhsT tiles (w[m][k] block)."""
    k, m = w.shape
    return np.ascontiguousarray(
        w.reshape(km, P, mm, P).transpose(2, 0, 1, 3)
    )


def _bf16(a):
    return np.ascontiguousarray(a).astype(ml_dtypes.bfloat16)


def _coltile(b, n):
    """[n*128] bias vector -> [128, n] column tiles (partition-major)."""
    return np.ascontiguousarray(np.asarray(b, np.float32).reshape(n, P).T)


# ---------------------------------------------------------------------------
# Device kernel builder
# ---------------------------------------------------------------------------

def _build(nc):
    xT = nc.dram_tensor("xT", [C, T], FP, kind="ExternalInput").ap()
    xTo = nc.dram_tensor("xTo", [C, TOWN], FP, kind="ExternalInput").ap()
    wq = nc.dram_tensor("wq", [NCT, NCT, P, P], FP, kind="ExternalInput").ap()
    wk = nc.dram_tensor("wk", [NCT, NCT, P, P], FP, kind="ExternalInput").ap()
    wv = nc.dram_tensor("wv", [NCT, NCT, P, P], FP, kind="ExternalInput").ap()
    wo = nc.dram_tensor("wo", [NCT, NCT, P, P], BF, kind="ExternalInput").ap()
    w1 = nc.dram_tensor("w1", [32, NCT, P, P], BF, kind="ExternalInput").ap()
    w2 = nc.dram_tensor("w2", [NCT, 32, P, P], BF, kind="ExternalInput").ap()
    gb = nc.dram_tensor("gb", [P, NCT, 4], FP, kind="ExternalInput").ap()
    bqkv = nc.dram_tensor("bqkv", [P, NCT, 3], FP, kind="ExternalInput").ap()
    b1m = nc.dram_tensor("b1m", [P, 32], FP, kind="ExternalInput").ap()
    msk = nc.dram_tensor("msk", [NQT, P, 512], FP, kind="ExternalInput").ap()
    outT = nc.dram_tensor("outT", [C, TOWN], FP, kind="ExternalOutput").ap()

    with tile.TileContext(nc) as tc:
        _build_tc(nc, tc, xT, xTo, wq, wk, wv, wo, w1, w2, gb, bqkv, b1m, msk,
                  outT)
    return nc


def _ln_stats(nc, psum_st, psum_bc, ln_sb, const, src, sl, ones_col, ones_row,
              eps_t, nb=2):
    """Channel-major LN stats for src[:, :, sl] ([128, NCT, 512] fp32r).
    Returns (rb, mb): [128, 512] fp32r broadcast tiles with
    rb = 1/std per token, mb = mean/std per token."""
    ssum = psum_st.tile([1, 512], FP, tag="ssum", bufs=1)
    ssq = psum_st.tile([1, 512], FP, tag="ssq", bufs=1)
    for ct in range(NCT):
        nc.tensor.matmul(ssum[:], ones_col[:], src[:, ct, sl],
                         start=(ct == 0), stop=(ct == NCT - 1))
    for ct in range(NCT):
        sq = ln_sb.tile([P, 512], FR, tag="sq", bufs=nb)
        nc.scalar.activation(sq[:], src[:, ct, sl], AF.Square)
        nc.tensor.matmul(ssq[:], ones_col[:], sq[:],
                         start=(ct == 0), stop=(ct == NCT - 1))
    mean = ln_sb.tile([1, 512], FP, tag="mean", bufs=nb)
    msq = ln_sb.tile([1, 512], FP, tag="msq", bufs=nb)
    nc.scalar.mul(mean[:], ssum[:], 1.0 / C)
    nc.scalar.mul(msq[:], ssq[:], 1.0 / C)
    var = ln_sb.tile([1, 512], FP, tag="var", bufs=nb)
    nc.vector.tensor_tensor(var[:], mean[:], mean[:], op=OP.mult)
    nc.vector.tensor_tensor(var[:], msq[:], var[:], op=OP.subtract)
    sd = ln_sb.tile([1, 512], FP, tag="sd", bufs=nb)
    nc.scalar.activation(sd[:], var[:], AF.Sqrt, bias=eps_t[0:1, :])
    rstd_f = ln_sb.tile([1, 512], FP, tag="rstdf", bufs=nb)
    nc.vector.reciprocal(rstd_f[:], sd[:])
    rstd = ln_sb.tile([1, 512], FR, tag="rstd", bufs=nb)
    nc.vector.tensor_copy(rstd[:], rstd_f[:])
    mrstd = ln_sb.tile([1, 512], FR, tag="mrstd", bufs=nb)
    nc.vector.tensor_tensor(mrstd[:], mean[:], rstd_f[:], op=OP.mult)
    rb_ps = psum_bc.tile([P, 512], FP, tag="bcps", bufs=1, name="rb_ps")
    nc.tensor.matmul(rb_ps[:], ones_row[:], rstd[:], start=True, stop=True)
    rb = ln_sb.tile([P, 512], FR, tag="rb", bufs=nb)
    nc.scalar.copy(rb[:], rb_ps[:])
    mb_ps = psum_bc.tile([P, 512], FP, tag="bcps", bufs=1, name="mb_ps")
    nc.tensor.matmul(mb_ps[:], ones_row[:], mrstd[:], start=True, stop=True)
    mb = ln_sb.tile([P, 512], FR, tag="mb", bufs=nb)
    nc.scalar.copy(mb[:], mb_ps[:])
    return rb, mb


def _build_tc(nc, tc, xT, xTo, wq, wk, wv, wo, w1, w2, gb, bqkv, b1m, msk,
              outT):
    const_cm = tc.tile_pool(name="const", bufs=1)
    const = const_cm.__enter__()
    ident = const.tile([P, P], BF)
    make_identity(nc, ident[:])
    ones_col = const.tile([P, 1], FR)
    nc.any.memset(ones_col[:].bitcast(FP), 1.0)
    ones_row = const.tile([1, P], FR)
    nc.any.memset(ones_row[:].bitcast(FP), 1.0)
    eps_t = const.tile([P, 1], FP)
    nc.any.memset(eps_t[:], LN_EPS)
    gb_t = const.tile([P, NCT, 4], FP)
    nc.sync.dma_start(gb_t[:], gb)
    bqkv_t = const.tile([P, NCT, 3], FP)
    nc.sync.dma_start(bqkv_t[:], bqkv)
    b1m_t = const.tile([P, 32], FP)
    nc.sync.dma_start(b1m_t[:], b1m)
    mask_t = const.tile([P, NQT, 512], FP)
    nc.sync.dma_start(mask_t[:], msk.rearrange("i p m -> p i m"))

    g1c, b1c = gb_t[:, :, 0], gb_t[:, :, 1]
    g2c, b2c = gb_t[:, :, 2], gb_t[:, :, 3]
    bq_c, bk_c, bv_c = bqkv_t[:, :, 0], bqkv_t[:, :, 1], bqkv_t[:, :, 2]

    pers_cm = tc.tile_pool(name="pers", bufs=1)
    pers = pers_cm.__enter__()
    xn_own = pers.tile([P, NCT, TOWN], FR)    # LN1 pre-gb, own tokens
    xn_gb = pers.tile([P, NCT, TOWN], FR)     # LN1 with g1/b1, own tokens
    out_t = pers.tile([P, NCT, TOWN], BF)     # attention output

    xnpool_cm = tc.tile_pool(name="xnpool", bufs=1)
    xnpool = xnpool_cm.__enter__()
    xn_t = xnpool.tile([P, NCT, T], FR)       # 8 MB: x^T -> ln1 pre-gb in place
    nc.sync.dma_start(xn_t[:], xT.rearrange("(ct p) t -> p ct t", p=P).bitcast(FR))
    nc.sync.dma_start(xn_own[:], xTo.rearrange("(ct p) t -> p ct t", p=P).bitcast(FR))

    # ---------------- LN1 (own tokens first, then the full batch) --------
    with tc.tile_pool(name="ln_sb", bufs=1) as ln_sb, \
         tc.tile_pool(name="ps_st", bufs=1, space="PSUM") as ps_st, \
         tc.tile_pool(name="ps_bc", bufs=1, space="PSUM") as ps_bc:
        rb, mb = _ln_stats(nc, ps_st, ps_bc, ln_sb, const, xn_own,
                           slice(0, 512), ones_col, ones_row, eps_t)
        for ct in range(NCT):
            t1 = ln_sb.tile([P, 512], FR, tag="t1", bufs=2)
            nc.vector.tensor_tensor(t1[:], xn_own[:, ct, :], rb[:], op=OP.mult)
            nc.vector.tensor_tensor(xn_own[:, ct, :], t1[:], mb[:], op=OP.subtract)
            nc.vector.tensor_scalar(
                xn_gb[:, ct], xn_own[:, ct], g1c[:, ct:ct + 1],
                b1c[:, ct:ct + 1], op0=OP.mult, op1=OP.add,
            )
        for ch in range(NTC):
            sl = slice(ch * 512, (ch + 1) * 512)
            rb, mb = _ln_stats(nc, ps_st, ps_bc, ln_sb, const, xn_t, sl,
                               ones_col, ones_row, eps_t)
            for ct in range(NCT):
                t1 = ln_sb.tile([P, 512], FR, tag="t1", bufs=2)
                nc.vector.tensor_tensor(t1[:], xn_t[:, ct, sl], rb[:], op=OP.mult)
                nc.vector.tensor_tensor(xn_t[:, ct, sl], t1[:], mb[:], op=OP.subtract)

    # ---------------- Interleaved projections + attention ----------------
    with tc.tile_pool(name="wpool", bufs=1) as wpool, \
         tc.tile_pool(name="kpool", bufs=2) as kpool, \
         tc.tile_pool(name="vpool", bufs=2) as vpool, \
         tc.tile_pool(name="qpool", bufs=2) as qpool, \
         tc.tile_pool(name="stpool", bufs=2) as stpool, \
         tc.tile_pool(name="attpool", bufs=1) as attpool, \
         tc.tile_pool(name="attsm", bufs=3) as attsm, \
         tc.tile_pool(name="psP", bufs=2, space="PSUM") as psP, \
         tc.tile_pool(name="psS", bufs=2, space="PSUM") as psS, \
         tc.tile_pool(name="psT", bufs=2, space="PSUM") as psT, \
         tc.tile_pool(name="psAV", bufs=2, space="PSUM") as psAV:
        for g in range(NG):
            # --- K projection for head pair g (all T tokens) ---
            wk_t = wpool.tile([P, NCT, P], FR, tag="wkt", bufs=2)
            nc.sync.dma_start(wk_t[:], wk[g].rearrange("k p m -> p k m").bitcast(FR))
            k_g = kpool.tile([P, T], FR, tag="kg")
            for ch in range(NTC):
                sl = slice(ch * 512, (ch + 1) * 512)
                ps = psP.tile([P, 512], FP, tag="psP")
                for k in range(NCT):
                    nc.tensor.matmul(ps[:], wk_t[:, k], xn_t[:, k, sl],
                                     start=(k == 0), stop=(k == NCT - 1))
                nc.vector.tensor_scalar(k_g[:, sl], ps[:], bk_c[:, g:g + 1], None,
                                        op0=OP.add)
            # --- V projection for head pair g, transposed to token-major ---
            wv_t = wpool.tile([P, NCT, P], FR, tag="wvt", bufs=2)
            nc.sync.dma_start(wv_t[:], wv[g].rearrange("k p m -> p k m").bitcast(FR))
            v_g = vpool.tile([P, T // P, P], BF, tag="vg")
            for ch in range(NTC):
                sl = slice(ch * 512, (ch + 1) * 512)
                ps = psP.tile([P, 512], FP, tag="psP")
                for k in range(NCT):
                    nc.tensor.matmul(ps[:], wv_t[:, k], xn_t[:, k, sl],
                                     start=(k == 0), stop=(k == NCT - 1))
                st = stpool.tile([P, 512], BF, tag="vst")
                nc.vector.tensor_scalar(st[:], ps[:], bv_c[:, g:g + 1], None,
                                        op0=OP.add)
                pst = psT.tile([P, 512], BF, tag="tps", name="vtp")
                for b4 in range(4):
                    nc.tensor.transpose(pst[:, b4 * P:(b4 + 1) * P],
                                        st[:, b4 * P:(b4 + 1) * P], ident[:])
                ev = v_g[:, ch * 4:(ch + 1) * 4, :].rearrange("p n d -> p (n d)")
                nc.scalar.copy(ev, pst[:])
            # --- Q projection for head pair g (own tokens, x64*g1 folded) ---
            wq_t = wpool.tile([P, NCT, P], FR, tag="wqt", bufs=2)
            nc.sync.dma_start(wq_t[:], wq[g].rearrange("k p m -> p k m").bitcast(FR))
            q_g = qpool.tile([P, TOWN], FR, tag="qg")
            ps = psP.tile([P, 512], FP, tag="psP")
            for k in range(NCT):
                nc.tensor.matmul(ps[:], wq_t[:, k], xn_own[:, k, :],
                                 start=(k == 0), stop=(k == NCT - 1))
            nc.vector.tensor_scalar(q_g[:], ps[:], bq_c[:, g:g + 1], None,
                                    op0=OP.add)

            # --- attention for head pair g ---
            for i in range(NQT):
                nch = i + 1
                attT = attpool.tile([P, T // P, 2 * P], BF, tag="attT", bufs=2)
                for h2 in range(2):
                    pb = h2 * 64
                    q_sl = q_g[pb:pb + 64, i * P:(i + 1) * P]
                    att = attpool.tile([P, T], BF, tag=f"att{h2}", bufs=2)
                    nmx = attsm.tile([P, NQT], FP, tag=f"nmx{h2}")
                    dsum = attsm.tile([P, NQT], FP, tag=f"dsum{h2}")
                    for kk in range(nch):
                        ps_s = psS.tile([P, 512], FP, tag="sps")
                        nc.tensor.matmul(ps_s[:], q_sl,
                                         k_g[pb:pb + 64, kk * 512:(kk + 1) * 512],
                                         start=True, stop=True)
                        if kk == i:
                            nc.vector.tensor_tensor(ps_s[:], ps_s[:],
                                                    mask_t[:, i, :], op=OP.add)
                        nc.vector.tensor_reduce(nmx[:, kk:kk + 1], ps_s[:],
                                                axis=AX.X, op=OP.max,
                                                negate=True)
                        nc.scalar.activation(att[:, kk * 512:(kk + 1) * 512],
                                             ps_s[:], AF.Exp,
                                             bias=nmx[:, kk:kk + 1],
                                             accum_out=dsum[:, kk:kk + 1])
                    # global max fixup + 1/den, folded into per-chunk scales
                    nM = attsm.tile([P, 1], FP, tag=f"nM{h2}")
                    nc.vector.tensor_reduce(nM[:], nmx[:, 0:nch], axis=AX.X,
                                            op=OP.min)
                    cvec = attsm.tile([P, NQT], FP, tag=f"cvec{h2}")
                    nc.scalar.activation(cvec[:, 0:nch], nmx[:, 0:nch], AF.Exp,
                                         bias=nM[:], scale=-1.0)
                    wden = attsm.tile([P, NQT], FP, tag=f"wden{h2}")
                    nc.vector.tensor_tensor(wden[:, 0:nch], dsum[:, 0:nch],
                                            cvec[:, 0:nch], op=OP.mult)
                    den = attsm.tile([P, 1], FP, tag=f"den{h2}")
                    nc.vector.tensor_reduce(den[:], wden[:, 0:nch], axis=AX.X,
                                            op=OP.add)
                    rden = attsm.tile([P, 1], FP, tag=f"rden{h2}")
                    nc.vector.reciprocal(rden[:], den[:])
                    svec = attsm.tile([P, NQT], FP, tag=f"svec{h2}")
                    nc.vector.tensor_scalar_mul(svec[:, 0:nch], cvec[:, 0:nch],
                                                rden[:])
                    # scale each chunk by svec_kk, then transpose
                    for kk in range(nch):
                        csl = slice(kk * 512, (kk + 1) * 512)
                        if kk % 2 == 0:
                            nc.vector.tensor_scalar_mul(att[:, csl], att[:, csl],
                                                        svec[:, kk:kk + 1])
                        else:
                            nc.scalar.mul(att[:, csl], att[:, csl],
                                          svec[:, kk:kk + 1])
                        ps_t = psT.tile([P, 512], BF, tag="tps")
                        for b4 in range(4):
                            blk = kk * 4 + b4
                            nc.tensor.transpose(ps_t[:, b4 * P:(b4 + 1) * P],
                                                att[:, blk * P:(blk + 1) * P],
                                                ident[:])
                        ev = attT[:, kk * 4:(kk + 1) * 4, h2 * P:(h2 + 1) * P]
                        src = ps_t[:].rearrange("p (n d) -> p n d", n=4)
                        if kk % 2 == 0:
                            nc.scalar.copy(ev, src)
                        else:
                            nc.vector.tensor_copy(ev, src)
                # AV: both heads' columns in one moving operand
                ps_av = psAV.tile([P, 2 * P], FP, tag="avps")
                for blk in range(nch * 4):
                    nc.tensor.matmul(
                        ps_av[:], v_g[:, blk, :], attT[:, blk, :],
                        start=(blk == 0), stop=(blk == nch * 4 - 1))
                nc.vector.tensor_copy(out_t[0:64, g, i * P:(i + 1) * P],
                                      ps_av[0:64, 0:P])
                nc.vector.tensor_copy(out_t[64:128, g, i * P:(i + 1) * P],
                                      ps_av[64:128, P:2 * P])

    xnpool_cm.__exit__(None, None, None)

    # ---------------- Tail: out-proj, LN2, MLP ---------------------------
    with tc.tile_pool(name="wpoolC", bufs=1) as wpoolC, \
         tc.tile_pool(name="ln_sbC", bufs=1) as ln_sbC, \
         tc.tile_pool(name="hpool", bufs=1) as hpool, \
         tc.tile_pool(name="apool", bufs=1) as apool, \
         tc.tile_pool(name="opool", bufs=2) as opool, \
         tc.tile_pool(name="psC", bufs=3, space="PSUM") as psC, \
         tc.tile_pool(name="psC_st", bufs=1, space="PSUM") as psC_st, \
         tc.tile_pool(name="psC_bc", bufs=1, space="PSUM") as psC_bc:
        h_t = hpool.tile([P, NCT, TOWN], FR)
        h2p = hpool.tile([P, NCT, TOWN], BF)     # LN2 pre-gb (for W1, g2 folded)
        h2gb = hpool.tile([P, NCT, TOWN], BF)    # LN2 with g2/b2 (residual)
        for m in range(NCT):
            wo_t = wpoolC.tile([P, NCT, P], BF, tag="wot", bufs=2)
            nc.sync.dma_start(wo_t[:], wo[m].rearrange("k p m -> p k m"))
            ps = psC.tile([P, 512], FP, tag="psC")
            for k in range(NCT):
                nc.tensor.matmul(ps[:], wo_t[:, k], out_t[:, k, :],
                                 start=(k == 0), stop=(k == NCT - 1))
            nc.vector.tensor_tensor(h_t[:, m], ps[:], xn_gb[:, m], op=OP.add)

        rb, mb = _ln_stats(nc, psC_st, psC_bc, ln_sbC, const, h_t,
                           slice(0, 512), ones_col, ones_row, eps_t, nb=1)
        for ct in range(NCT):
            t1 = ln_sbC.tile([P, 512], FR, tag="t1", bufs=2)
            nc.vector.tensor_tensor(t1[:], h_t[:, ct, :], rb[:], op=OP.mult)
            nc.vector.tensor_tensor(h2p[:, ct], t1[:], mb[:], op=OP.subtract)
            nc.vector.tensor_scalar(
                h2gb[:, ct], h2p[:, ct], g2c[:, ct:ct + 1], b2c[:, ct:ct + 1],
                op0=OP.mult, op1=OP.add,
            )

        a_t = apool.tile([P, 32, TOWN], BF)
        for m in range(32):
            w1_t = wpoolC.tile([P, NCT, P], BF, tag="w1t", bufs=3)
            nc.sync.dma_start(w1_t[:], w1[m].rearrange("k p m -> p k m"))
            ps = psC.tile([P, 512], FP, tag="psC")
            for k in range(NCT):
                nc.tensor.matmul(ps[:], w1_t[:, k], h2p[:, k, :],
                                 start=(k == 0), stop=(k == NCT - 1))
            nc.vector.tensor_scalar(a_t[:, m], ps[:], b1m_t[:, m:m + 1], 0.0,
                                    op0=OP.add, op1=OP.max)

        for m in range(NCT):
            w2_t = wpoolC.tile([P, 32, P], BF, tag="w2t", bufs=2)
            nc.sync.dma_start(w2_t[:], w2[m].rearrange("k p m -> p k m"))
            ps = psC.tile([P, 512], FP, tag="psC")
            for k in range(32):
                nc.tensor.matmul(ps[:], w2_t[:, k], a_t[:, k, :],
                                 start=(k == 0), stop=(k == 31))
            o_m = opool.tile([P, 512], FP, tag="om")
            nc.vector.tensor_tensor(o_m[:], ps[:], h2gb[:, m], op=OP.add)
            nc.sync.dma_start(outT[m * P:(m + 1) * P, :], o_m[:])

    pers_cm.__exit__(None, None, None)
    const_cm.__exit__(None, None, None)


# ---------------------------------------------------------------------------
# Public entry point
# ---------------------------------------------------------------------------
_cache = {}


def _get_nc():
    if "nc" not in _cache:
        _apply_tile_patch()
        nc = bass.Bass("TRN2", target_bir_lowering=False, debug=False,
                       num_devices=8)
        _build(nc)
        _cache["nc"] = nc
    return _cache["nc"]


def run(inputs, trace=False):
    x = np.asarray(inputs["x"], np.float32)
    Wk = np.asarray(inputs["Wk"], np.float32)
    Wq = np.asarray(inputs["Wq"], np.float32)
    Wv = np.asarray(inputs["Wv"], np.float32)
    Wo = np.asarray(inputs["Wo"], np.float32)
    W1 = np.asarray(inputs["W1"], np.float32)
    W2 = np.asarray(inputs["W2"], np.float32)
    g1 = np.asarray(inputs["g1"], np.float32)
    b1 = np.asarray(inputs["b1"], np.float32)
    g2 = np.asarray(inputs["g2"], np.float32)
    b2 = np.asarray(inputs["b2"], np.float32)

    # Fold LN gains into the weights; LN biases become per-output-column
    # bias rows added at PSUM eviction.
    wq_t = _r12(_lhsT_tiles(_r12(g1[:, None] * Wq * float(HD)), NCT, NCT))
    wk_t = _r12(_lhsT_tiles(_r12(g1[:, None] * Wk), NCT, NCT))
    wv_t = _r12(_lhsT_tiles(_r12(g1[:, None] * Wv), NCT, NCT))
    wo_t = _bf16(_lhsT_tiles(Wo, NCT, NCT))
    w1_t = _bf16(_lhsT_tiles(g2[:, None] * W1, NCT, 32))
    w2_t = _bf16(_lhsT_tiles(W2, 32, NCT))
    bq = _coltile(float(HD) * (b1 @ Wq), NCT)
    bk = _coltile(b1 @ Wk, NCT)
    bv = _coltile(b1 @ Wv, NCT)
    bqkv = np.ascontiguousarray(np.stack([bq, bk, bv], axis=-1))  # [P, NCT, 3]
    b1m_h = _coltile(b2 @ W1, 32)
    gbh = np.stack(
        [g1.reshape(NCT, P).T, b1.reshape(NCT, P).T,
         g2.reshape(NCT, P).T, b2.reshape(NCT, P).T], axis=-1
    ).astype(np.float32)  # [P, NCT, 4]

    in_maps = []
    own_tokens_by_core = []
    for c in range(8):
        b = c // 4
        j = c % 4
        tiles = [j + 4 * i for i in range(NQT)]
        toks = np.concatenate([np.arange(t * P, (t + 1) * P) for t in tiles])
        own_tokens_by_core.append((b, toks))
        xT_full = _r12(np.ascontiguousarray(x[b].T))
        xT_own = _r12(np.ascontiguousarray(x[b][toks].T))
        mask = np.zeros((NQT, P, 512), np.float32)
        for i in range(NQT):
            t0 = (j + 4 * i) * P
            Ei = (i + 1) * 512
            cols = (Ei - 512) + np.arange(512)
            rows = t0 + np.arange(P)
            mask[i] = np.where(cols[None, :] <= rows[:, None], 0.0, -1.0e30)
        in_maps.append({
            "xT": xT_full, "xTo": xT_own,
            "wq": wq_t, "wk": wk_t, "wv": wv_t, "wo": wo_t,
            "w1": w1_t, "w2": w2_t, "gb": gbh,
            "bqkv": bqkv, "b1m": b1m_h, "msk": mask,
        })

    nc = _get_nc()
    res = run_bass_kernel_spmd(nc, in_maps, core_ids=list(range(8)),
                               trace=trace)

    out = np.empty((B, T, C), np.float32)
    for c in range(8):
        b, toks = own_tokens_by_core[c]
        out[b, toks, :] = res.results[c]["outT"].T
    return out, res


def kernel(**inputs):
    out, _ = run(inputs, trace=False)
    return out
